# revision 19
# baseline (speedup 1.0000x reference)
"""GCN (3-layer + encoder + global-add-pool + MLP readout) on 8 Trainium2 NeuronCores.

Strategy (dst-partitioned message passing):
  - Nodes are permuted into 8 shards x 98 blocks x 128 slots (degree-balanced
    blocks so every (block, chunk) edge bucket fits a uniform tile count).
  - Each layer: h_out = relu(AGG(h_in) @ W + b), where AGG is the normalized
    adjacency aggregation (self-loops folded in as ordinary edges with weight
    dinv^2).  The encoder folds into layer 1: gather x directly and use
    W_enc@W1 plus a rank-1 nsum x (b_enc@W1) bias correction.
  - Per block: dma_gather pulls the 128-row edge-source tiles (bf16) from the
    full replicated node table; DVE builds S = (iota == dst_local) * w; PE
    accumulates V^T @ S into PSUM (feature-major agg); then agg @ W + bias and
    relu -> shard.  AllGather replicates shards between layers.
  - Layer 3 output pools straight from SBUF into a persistent PSUM [d, G]
    accumulator via one-hot batch matmuls; AllReduce + tiny MLP readout.
"""

import numpy as np
import ml_dtypes
from dataclasses import dataclass

import concourse.bass as bass
import concourse.bacc as bacc
import concourse.mybir as mybir
import concourse.tile as tile
from concourse.bass import AP
from concourse.bass_utils import run_bass_kernel_spmd

BF16 = ml_dtypes.bfloat16
F32 = np.float32
DT = mybir.dt


@dataclass(frozen=True)
class Cfg:
    n_nodes: int
    n_edges: int
    n_graphs: int
    d: int
    n_classes: int
    p: int                # cores
    blocks_per_core: int
    bpg: int              # blocks per gather group
    nchunk: int = 4

    @property
    def groups(self):
        assert self.blocks_per_core % self.bpg == 0
        return self.blocks_per_core // self.bpg

    @property
    def slots(self):
        return self.blocks_per_core * 128

    @property
    def rows(self):
        return self.p * self.slots

    @property
    def chunk(self):
        assert self.rows % self.nchunk == 0
        return self.rows // self.nchunk


REAL_CFG = Cfg(n_nodes=100000, n_edges=1600000, n_graphs=512, d=128,
               n_classes=10, p=8, blocks_per_core=98, bpg=7)


# ---------------------------------------------------------------- host side

class Layout:
    """Shared host/device addressing for the variable-tile-count layout.

    Tmat[b, c] = tiles for (local block b, chunk c); identical across cores.
    """

    def __init__(self, cfg: Cfg, Tmat):
        self.cfg = cfg
        self.Tmat = np.asarray(Tmat, np.int64)
        assert self.Tmat.shape == (cfg.blocks_per_core, cfg.nchunk)
        self.cap = self.Tmat * 128
        self.NTb = self.Tmat.sum(axis=1)                       # [BPC]
        self.coff = np.zeros_like(self.Tmat)
        self.coff[:, 1:] = np.cumsum(self.Tmat, axis=1)[:, :-1]

        G, B = cfg.groups, cfg.bpg
        # gather call (g, c): tiles per call and per-block column offsets
        self.vtiles = np.zeros((G, cfg.nchunk), np.int64)
        self.voff = np.zeros((G, cfg.nchunk, B), np.int64)
        for g in range(G):
            for c in range(cfg.nchunk):
                ts = self.Tmat[g * B:(g + 1) * B, c]
                self.voff[g, c] = np.concatenate([[0], np.cumsum(ts)[:-1]])
                self.vtiles[g, c] = ts.sum()
        # idx array columns (16-wrapped): per (g, c) slice offsets
        self.idx_cols = self.vtiles * 8                        # tiles*128/16
        self.idx_off = np.zeros((G, cfg.nchunk), np.int64)
        self.idx_off[:, 1:] = np.cumsum(self.idx_cols, axis=1)[:, :-1]
        self.idx_cols_total = int(self.idx_cols.sum(axis=1).max())
        # operand (dstl/w) columns: block-major within a group
        self.op_off = np.zeros(cfg.blocks_per_core, np.int64)
        for g in range(G):
            nt = self.NTb[g * B:(g + 1) * B]
            self.op_off[g * B:(g + 1) * B] = (
                np.concatenate([[0], np.cumsum(nt)[:-1]]))
        self.op_cols_total = int(
            self.NTb.reshape(G, B).sum(axis=1).max())
        self.vtiles_max = int(self.vtiles.max())
        self.NTb_max = int(self.NTb.max())

    def key(self):
        return self.Tmat.tobytes()


def preprocess(cfg: Cfg, x, src, dst, batch):
    """Compute the node permutation, normalization constants and the padded
    per-core edge streams.  Returns (tc, per_core_meta, shared)."""
    N, E, P = cfg.n_nodes, cfg.n_edges, cfg.p
    NB = P * cfg.blocks_per_core              # global block count
    CH = cfg.chunk

    deg = np.bincount(dst, minlength=N).astype(np.float64) + 1.0
    dinv = 1.0 / np.sqrt(deg)
    norm = (dinv[src] * dinv[dst]).astype(np.float64)
    dinv2 = dinv * dinv
    nsum = np.bincount(dst, weights=norm, minlength=N) + dinv2

    # --- serpentine degree-balanced assignment of nodes to global blocks
    order = np.argsort(-deg, kind="stable")
    k = np.arange(N)
    rowi, coli = k // NB, k % NB
    blk_ser = np.where(rowi % 2 == 0, coli, NB - 1 - coli)
    blk_of = np.empty(N, np.int64)
    blk_of[order] = blk_ser
    # slot index within block: stable order of appearance
    o2 = np.argsort(blk_of, kind="stable")
    slot_idx = np.empty(N, np.int64)
    counts = np.bincount(blk_of, minlength=NB)
    assert counts.max() <= 128
    starts = np.concatenate([[0], np.cumsum(counts)[:-1]])
    slot_idx[o2] = np.arange(N) - starts[blk_of[o2]]
    row_of = blk_of * 128 + slot_idx                  # table row of each node

    # --- edge items (graph edges + self loops), keyed by (block, chunk)
    eb = blk_of[dst]
    ei = row_of[src]
    items_key = eb * cfg.nchunk + (ei // CH)
    items_idx = (ei % CH).astype(np.int64)
    items_dstl = (row_of[dst] % 128).astype(np.int64)
    items_w = norm

    sb = blk_of  # self loops: one per node
    si = row_of
    skey = sb * cfg.nchunk + (si // CH)
    key = np.concatenate([items_key, skey])
    idxv = np.concatenate([items_idx, si % CH])
    dstlv = np.concatenate([items_dstl, row_of % 128])
    wv = np.concatenate([items_w, dinv2])

    nbuck = NB * cfg.nchunk
    cnt = np.bincount(key, minlength=nbuck)

    # per-(local block, chunk) tile counts, maxed across cores so the SPMD
    # program structure is uniform
    cnt3 = cnt.reshape(P, cfg.blocks_per_core, cfg.nchunk)
    Tmat = np.ceil(cnt3.max(axis=0) / 128).astype(np.int64)   # [BPC, nchunk]
    Tmat = np.maximum(Tmat, 1)
    lay = Layout(cfg, Tmat)

    # position of each item within its bucket
    o3 = np.argsort(key, kind="stable")
    bstart = np.concatenate([[0], np.cumsum(cnt)[:-1]])
    pos = np.arange(key.size) - bstart[key[o3]]

    # scatter items into the padded per-bucket layout (flat address space)
    cap_flat = np.tile(lay.cap.reshape(-1), P)           # [nbuck]
    boff = np.concatenate([[0], np.cumsum(cap_flat)[:-1]])
    addr = boff[key[o3]] + pos
    tot = int(cap_flat.sum())
    flat_idx = np.zeros(tot, np.int16)
    flat_dstl = np.full(tot, -1.0, np.float32)
    flat_w = np.zeros(tot, np.float32)
    flat_idx[addr] = idxv[o3].astype(np.int16)
    flat_dstl[addr] = dstlv[o3].astype(np.float32)
    flat_w[addr] = wv[o3].astype(np.float32)
    per_core_tot = tot // P
    flat_idx = flat_idx.reshape(P, per_core_tot)
    flat_dstl = flat_dstl.reshape(P, per_core_tot)
    flat_w = flat_w.reshape(P, per_core_tot)

    # bucket start offsets within one core's flat stream, [BPC, nchunk]
    bko = boff[:nbuck // P].reshape(cfg.blocks_per_core, cfg.nchunk)

    per_core = []
    for r in range(P):
        b0 = r * cfg.blocks_per_core
        b1 = b0 + cfg.blocks_per_core
        # gather index streams: [groups, 128, sum(call cols)] int16
        gi = np.zeros((cfg.groups, 128, lay.idx_cols_total), np.int16)
        dl = np.full((cfg.groups, 128, lay.op_cols_total), -1.0, np.float32)
        wl_ = np.zeros((cfg.groups, 128, lay.op_cols_total), np.float32)
        for g in range(cfg.groups):
            for c in range(cfg.nchunk):
                parts = []
                for bl in range(cfg.bpg):
                    b = g * cfg.bpg + bl
                    o = bko[b, c]
                    parts.append(flat_idx[r, o:o + lay.cap[b, c]])
                stream = np.concatenate(parts)
                w16 = stream.reshape(-1, 16).T            # wrap 16 partitions
                j0 = lay.idx_off[g][c]
                gi[g, :, j0:j0 + w16.shape[1]] = np.tile(w16, (8, 1))
            for bl in range(cfg.bpg):
                b = g * cfg.bpg + bl
                for c in range(cfg.nchunk):
                    o = bko[b, c]
                    T = lay.Tmat[b, c]
                    blk_d = flat_dstl[r, o:o + T * 128].reshape(T, 128).T
                    blk_w = flat_w[r, o:o + T * 128].reshape(T, 128).T
                    j0 = lay.op_off[b] + lay.coff[b, c]
                    dl[g, :, j0:j0 + T] = blk_d
                    wl_[g, :, j0:j0 + T] = blk_w
        wl_ = wl_.astype(BF16)

        # batch_local [128, BPC] and nsum_ones [2, SLOTS]
        bl = np.full((cfg.blocks_per_core, 128), -1.0, np.float32)
        ns = np.zeros((cfg.blocks_per_core, 128), np.float32)
        nodes_r = np.where((blk_of >= b0) & (blk_of < b1))[0]
        lb = blk_of[nodes_r] - b0
        sl = slot_idx[nodes_r]
        bl[lb, sl] = batch[nodes_r].astype(np.float32)
        ns[lb, sl] = nsum[nodes_r].astype(np.float32)
        batchl = bl.T.copy()                          # [128, BPC]
        nsum1 = np.stack([ns.reshape(-1),
                          np.ones(cfg.slots, np.float32)])  # [2, SLOTS]
        per_core.append(dict(idx=gi, dstl=dl, wgt=wl_, batchl=batchl,
                             nsum1=nsum1))

    xt = np.zeros((cfg.rows, cfg.d), BF16)
    xt[row_of] = x.astype(BF16)
    return lay, per_core, dict(xt=xt, row_of=row_of)


def make_consts(cfg: Cfg, W_enc, b_enc, W1, b1, W2, b2, W3, b3, Wr1, br1,
                Wr2, br2):
    d = cfg.d
    wf1 = (W_enc.astype(np.float64) @ W1.astype(np.float64)).astype(F32)
    bw = (b_enc.astype(np.float64) @ W1.astype(np.float64)).astype(F32)
    wl = np.stack([wf1, W2.astype(F32), W3.astype(F32)])          # [3,d,d]
    brhs = np.zeros((3, 2, d), F32)
    brhs[0, 0] = bw
    brhs[0, 1] = b1
    brhs[1, 1] = b2
    brhs[2, 1] = b3
    iota128 = np.tile(np.arange(128, dtype=F32), (128, 1))
    iotag = np.tile(np.arange(cfg.n_graphs, dtype=F32), (128, 1))
    return dict(
        wl=wl, brhs=brhs, iota128=iota128, iotag=iotag,
        wr1=Wr1.astype(F32), br1=br1.astype(F32).reshape(d, 1),
        wr2=Wr2.astype(F32), br2=br2.astype(F32).reshape(cfg.n_classes, 1),
    )


# ---------------------------------------------------------------- device side

def build_program(cfg: Cfg, lay: Layout, enable_asserts=False, debug=False,
                  maxt=8, no_gather=False, nq=4):
    d = cfg.d
    G = cfg.n_graphs
    BPC, GR, BPGR = cfg.blocks_per_core, cfg.groups, cfg.bpg
    Tm, coff, NTb = lay.Tmat, lay.coff, lay.NTb

    nc = bacc.Bacc("TRN2", target_bir_lowering=False, debug=debug,
                   enable_asserts=enable_asserts, num_devices=cfg.p,
                   num_swdge_queues=nq)

    ti = lambda n, s, t: nc.dram_tensor(n, s, t, kind="ExternalInput")
    xt_t = ti("xt", [cfg.rows, d], DT.bfloat16)
    idx_t = ti("idx", [GR, 128, lay.idx_cols_total], DT.int16)
    dstl_t = ti("dstl", [GR, 128, lay.op_cols_total], DT.float32)
    wgt_t = ti("wgt", [GR, 128, lay.op_cols_total], DT.bfloat16)
    batchl_t = ti("batchl", [128, BPC], DT.float32)
    nsum1_t = ti("nsum1", [2, cfg.slots], DT.float32)
    wl_t = ti("wl", [3, d, d], DT.float32)
    brhs_t = ti("brhs", [3, 2, d], DT.float32)
    iota128_t = ti("iota128", [128, 128], DT.float32)
    iotag_t = ti("iotag", [128, G], DT.float32)
    wr1_t = ti("wr1", [d, d], DT.float32)
    br1_t = ti("br1", [d, 1], DT.float32)
    wr2_t = ti("wr2", [d, cfg.n_classes], DT.float32)
    br2_t = ti("br2", [cfg.n_classes, 1], DT.float32)

    logits_t = nc.dram_tensor("logits", [cfg.n_classes, G], DT.float32,
                              kind="ExternalOutput")

    # internal DRAM
    shard = [nc.dram_tensor(f"shard{l}", [cfg.slots, d], DT.bfloat16)
             for l in range(2)]
    tbl = [nc.dram_tensor(f"tbl{l}", [cfg.rows, d], DT.bfloat16,
                          addr_space="Shared") for l in range(2)]
    pool_in = nc.dram_tensor("pool_in", [d, G], DT.float32)
    pool_out = nc.dram_tensor("pool_out", [d, G], DT.float32,
                              addr_space="Shared")

    rg = [list(range(cfg.p))]

    def bcast3(ap2, reps, inner):
        """[P, C] AP -> [P, C, inner] AP with a step-0 inner dim (reps = C)."""
        a = ap2
        assert len(a.ap) == 2
        return AP(a.tensor, a.offset, [a.ap[0], [a.ap[1][0], reps], [0, inner]])

    with tile.TileContext(nc) as tc:
        with (
            tc.tile_pool(name="const", bufs=1) as cpool,
            tc.tile_pool(name="meta", bufs=3) as mpool,
            tc.tile_pool(name="v0", bufs=2) as vp0,
            tc.tile_pool(name="v1", bufs=2) as vp1,
            tc.tile_pool(name="v2", bufs=2) as vp2,
            tc.tile_pool(name="v3", bufs=2) as vp3,
            tc.tile_pool(name="s", bufs=2) as spool,
            tc.tile_pool(name="work", bufs=2) as wpool,
            tc.tile_pool(name="rout", bufs=1) as routp,
            tc.tile_pool(name="agg", bufs=2, space="PSUM") as aggp,
            tc.tile_pool(name="hps", bufs=2, space="PSUM") as hpsp,
            tc.tile_pool(name="poolacc", bufs=1, space="PSUM") as plp,
            tc.tile_pool(name="ro", bufs=1, space="PSUM") as rop,
        ):
            vpools = [vp0, vp1, vp2, vp3]

            # ---- resident constants / metadata
            iota = cpool.tile([128, 128], DT.float32)
            nc.sync.dma_start(iota[:], iota128_t[:])
            iotag = cpool.tile([128, G], DT.float32)
            nc.sync.dma_start(iotag[:], iotag_t[:])
            wl = cpool.tile([128, 3, d], DT.float32)
            nc.sync.dma_start(wl[:], wl_t.rearrange("l k j -> k l j"))
            brhs = cpool.tile([2, 3, d], DT.float32)
            nc.sync.dma_start(brhs[:], brhs_t.rearrange("l k j -> k l j"))
            wr1 = cpool.tile([d, d], DT.float32)
            nc.sync.dma_start(wr1[:], wr1_t[:])
            br1 = cpool.tile([d, 1], DT.float32)
            nc.sync.dma_start(br1[:], br1_t[:])
            wr2 = cpool.tile([d, cfg.n_classes], DT.float32)
            nc.sync.dma_start(wr2[:], wr2_t[:])
            br2 = cpool.tile([cfg.n_classes, 1], DT.float32)
            nc.sync.dma_start(br2[:], br2_t[:])
            batchl = cpool.tile([128, BPC], DT.float32)
            nc.sync.dma_start(batchl[:], batchl_t[:])
            nsum1 = cpool.tile([2, cfg.slots], DT.float32)
            nc.sync.dma_start(nsum1[:], nsum1_t[:])

            pool_ps = plp.tile([d, G], DT.float32)

            for l in range(3):
                table = xt_t if l == 0 else tbl[l - 1]
                for g in range(GR):
                    idxg = mpool.tile([128, lay.idx_cols_total], DT.int16,
                                      tag="idxg")
                    nc.sync.dma_start(idxg[:], idx_t[g])
                    dstlg = mpool.tile([128, lay.op_cols_total], DT.float32,
                                       tag="dstlg")
                    nc.sync.dma_start(dstlg[:], dstl_t[g])
                    wgtg = mpool.tile([128, lay.op_cols_total], DT.bfloat16,
                                      tag="wgtg")
                    nc.sync.dma_start(wgtg[:], wgt_t[g])
                    vt = []
                    for c in range(cfg.nchunk):
                        nv = int(lay.vtiles[g, c])
                        v = vpools[c].tile([128, lay.vtiles_max, d],
                                           DT.bfloat16, tag=f"v{c}")
                        i0 = int(lay.idx_off[g, c])
                        if no_gather:
                            nc.vector.memset(v[:], 0.5)
                        # SWDGE descriptor-ring carveout caps one gather at
                        # <4096 indices; split into sub-calls of <=maxt tiles.
                        for j0 in range(0, 0 if no_gather else nv, maxt):
                            j1 = min(j0 + maxt, nv)
                            nc.gpsimd.dma_gather(
                                out_ap=v[:, j0:j1, :],
                                in_ap=table[c * cfg.chunk:
                                            (c + 1) * cfg.chunk, :],
                                idxs_ap=idxg[:, i0 + j0 * 8:i0 + j1 * 8],
                                num_idxs=(j1 - j0) * 128,
                                num_idxs_reg=(j1 - j0) * 128,
                                elem_size=d,
                                queue_num=c % nq,
                            )
                        vt.append(v)
                    for bl in range(BPGR):
                        b = g * BPGR + bl
                        nt = int(NTb[b])
                        col0 = int(lay.op_off[b])
                        # S = (iota == dstl) * w   [128, nt*128] bf16
                        s01 = spool.tile([128, lay.NTb_max * 128],
                                         DT.bfloat16, tag="s01")
                        s = spool.tile([128, lay.NTb_max * 128],
                                       DT.bfloat16, tag="s")
                        io3 = AP(iota[:].tensor, iota[:].offset,
                                 [iota[:].ap[0], [0, nt], iota[:].ap[1]])
                        d2 = dstlg[:, col0:col0 + nt]
                        w2 = wgtg[:, col0:col0 + nt]
                        s3 = s[:, :nt * 128].rearrange(
                            "p (t j) -> p t j", j=128)
                        s013 = s01[:, :nt * 128].rearrange(
                            "p (t j) -> p t j", j=128)
                        nc.vector.tensor_tensor(
                            out=s013, in0=io3, in1=bcast3(d2, nt, 128),
                            op=mybir.AluOpType.is_equal)
                        nc.vector.tensor_tensor(
                            out=s3, in0=s013, in1=bcast3(w2, nt, 128),
                            op=mybir.AluOpType.mult)
                        agg = aggp.tile([d, 128], DT.float32)
                        for c in range(cfg.nchunk):
                            for t in range(int(Tm[b, c])):
                                kk = int(coff[b, c]) + t
                                nc.tensor.matmul(
                                    agg[:],
                                    lhsT=vt[c][:, int(lay.voff[g, c, bl]) + t, :],
                                    rhs=s[:, kk * 128:(kk + 1) * 128],
                                    start=(kk == 0), stop=(kk == nt - 1))
                        aggs = wpool.tile([d, 128], DT.float32, tag="aggs")
                        nc.scalar.activation(aggs[:], agg[:],
                                             mybir.ActivationFunctionType.Copy)
                        hps = hpsp.tile([128, d], DT.float32)
                        nc.tensor.matmul(hps[:], lhsT=aggs[:],
                                         rhs=wl[:, l, :],
                                         start=True, stop=False)
                        nc.tensor.matmul(
                            hps[:], lhsT=nsum1[:, b * 128:(b + 1) * 128],
                            rhs=brhs[:, l, :], start=False, stop=True)
                        if l < 2:
                            hout = wpool.tile([128, d], DT.bfloat16,
                                              tag="hout")
                            nc.scalar.activation(
                                hout[:], hps[:],
                                mybir.ActivationFunctionType.Relu)
                            nc.sync.dma_start(
                                shard[l][b * 128:(b + 1) * 128, :], hout[:])
                        else:
                            h3 = wpool.tile([128, d], DT.float16, tag="h3")
                            nc.scalar.activation(
                                h3[:], hps[:],
                                mybir.ActivationFunctionType.Relu)
                            sp = spool.tile([128, G], DT.float16, tag="sp")
                            nc.vector.tensor_scalar(
                                out=sp[:], in0=iotag[:],
                                scalar1=batchl[:, b:b + 1], scalar2=None,
                                op0=mybir.AluOpType.is_equal)
                            nc.tensor.matmul(pool_ps[:], lhsT=h3[:],
                                             rhs=sp[:], start=(b == 0),
                                             stop=(b == BPC - 1))
                if l < 2:
                    nc.gpsimd.collective_compute(
                        "AllGather", mybir.AluOpType.bypass,
                        replica_groups=rg,
                        ins=[shard[l][:]], outs=[tbl[l][:]])

            # ---- pooled readout
            pools = routp.tile([d, G], DT.float32, tag="pools")
            nc.scalar.activation(pools[:], pool_ps[:],
                                 mybir.ActivationFunctionType.Copy)
            nc.sync.dma_start(pool_in[:], pools[:])
            nc.gpsimd.collective_compute(
                "AllReduce", mybir.AluOpType.add, replica_groups=rg,
                ins=[pool_in[:]], outs=[pool_out[:]])
            gfm = routp.tile([d, G], DT.float32, tag="gfm")
            nc.sync.dma_start(gfm[:], pool_out[:])
            z1p = rop.tile([d, G], DT.float32, tag="ro")
            nc.tensor.matmul(z1p[:], lhsT=wr1[:], rhs=gfm[:],
                             start=True, stop=True)
            z1 = routp.tile([d, G], DT.float32, tag="z1")
            nc.scalar.activation(z1[:], z1p[:],
                                 mybir.ActivationFunctionType.Relu,
                                 bias=br1[:])
            lgp = rop.tile([cfg.n_classes, G], DT.float32, tag="ro")
            nc.tensor.matmul(lgp[:], lhsT=wr2[:], rhs=z1[:],
                             start=True, stop=True)
            lg = routp.tile([cfg.n_classes, G], DT.float32, tag="lg")
            nc.scalar.activation(lg[:], lgp[:],
                                 mybir.ActivationFunctionType.Identity,
                                 bias=br2[:])
            nc.sync.dma_start(logits_t[:], lg[:])

    nc.compile()
    return nc


# ---------------------------------------------------------------- entry point

_PROG_CACHE = {}


def _get_program(cfg, lay, **kw):
    key = (cfg, lay.key(), tuple(sorted(kw.items())))
    if key not in _PROG_CACHE:
        _PROG_CACHE[key] = build_program(cfg, lay, **kw)
    return _PROG_CACHE[key]


def make_in_maps(cfg, inputs):
    x = np.asarray(inputs["x"], np.float32)
    src = np.asarray(inputs["c_2"], np.int64)
    dst = np.asarray(inputs["u_2"], np.int64)
    batch = np.asarray(inputs["batch"], np.int64)
    lay, per_core, shared = preprocess(cfg, x, src, dst, batch)
    consts = make_consts(
        cfg, *(np.asarray(inputs[k], np.float32) for k in
               ("W_enc", "b_enc", "W1", "b1", "W2", "b2", "W3", "b3",
                "Wr1", "br1", "Wr2", "br2")))
    in_maps = []
    for r in range(cfg.p):
        m = dict(per_core[r])
        m["xt"] = shared["xt"]
        m.update(consts)
        in_maps.append(m)
    return lay, in_maps


def kernel(**inputs) -> np.ndarray:
    cfg = REAL_CFG
    lay, in_maps = make_in_maps(cfg, inputs)
    nc = _get_program(cfg, lay)
    res = run_bass_kernel_spmd(nc, in_maps, list(range(cfg.p)))
    logits = np.asarray(res.results[0]["logits"], np.float32)
    return np.ascontiguousarray(logits.T)


# revision 20
# speedup vs baseline: 1.0019x; 1.0019x over previous
"""GCN (3-layer + encoder + global-add-pool + MLP readout) on 8 Trainium2 NeuronCores.

Strategy (dst-partitioned message passing):
  - Nodes are permuted into 8 shards x 98 blocks x 128 slots (degree-balanced
    blocks so every (block, chunk) edge bucket fits a uniform tile count).
  - Each layer: h_out = relu(AGG(h_in) @ W + b), where AGG is the normalized
    adjacency aggregation (self-loops folded in as ordinary edges with weight
    dinv^2).  The encoder folds into layer 1: gather x directly and use
    W_enc@W1 plus a rank-1 nsum x (b_enc@W1) bias correction.
  - Per block: dma_gather pulls the 128-row edge-source tiles (bf16) from the
    full replicated node table; DVE builds S = (iota == dst_local) * w; PE
    accumulates V^T @ S into PSUM (feature-major agg); then agg @ W + bias and
    relu -> shard.  AllGather replicates shards between layers.
  - Layer 3 output pools straight from SBUF into a persistent PSUM [d, G]
    accumulator via one-hot batch matmuls; AllReduce + tiny MLP readout.
"""

import numpy as np
import ml_dtypes
from dataclasses import dataclass

import concourse.bass as bass
import concourse.bacc as bacc
import concourse.mybir as mybir
import concourse.tile as tile
from concourse.bass import AP
from concourse.bass_utils import run_bass_kernel_spmd

BF16 = ml_dtypes.bfloat16
F32 = np.float32
DT = mybir.dt


@dataclass(frozen=True)
class Cfg:
    n_nodes: int
    n_edges: int
    n_graphs: int
    d: int
    n_classes: int
    p: int                # cores
    blocks_per_core: int
    bpg: int              # blocks per gather group
    nchunk: int = 4

    @property
    def groups(self):
        assert self.blocks_per_core % self.bpg == 0
        return self.blocks_per_core // self.bpg

    @property
    def slots(self):
        return self.blocks_per_core * 128

    @property
    def rows(self):
        return self.p * self.slots

    @property
    def chunk(self):
        assert self.rows % self.nchunk == 0
        return self.rows // self.nchunk


REAL_CFG = Cfg(n_nodes=100000, n_edges=1600000, n_graphs=512, d=128,
               n_classes=10, p=8, blocks_per_core=98, bpg=7)


# ---------------------------------------------------------------- host side

class Layout:
    """Shared host/device addressing for the variable-tile-count layout.

    Tmat[b, c] = tiles for (local block b, chunk c); identical across cores.
    """

    def __init__(self, cfg: Cfg, Tmat):
        self.cfg = cfg
        self.Tmat = np.asarray(Tmat, np.int64)
        assert self.Tmat.shape == (cfg.blocks_per_core, cfg.nchunk)
        self.cap = self.Tmat * 128
        self.NTb = self.Tmat.sum(axis=1)                       # [BPC]
        self.coff = np.zeros_like(self.Tmat)
        self.coff[:, 1:] = np.cumsum(self.Tmat, axis=1)[:, :-1]

        G, B = cfg.groups, cfg.bpg
        # gather call (g, c): tiles per call and per-block column offsets
        self.vtiles = np.zeros((G, cfg.nchunk), np.int64)
        self.voff = np.zeros((G, cfg.nchunk, B), np.int64)
        for g in range(G):
            for c in range(cfg.nchunk):
                ts = self.Tmat[g * B:(g + 1) * B, c]
                self.voff[g, c] = np.concatenate([[0], np.cumsum(ts)[:-1]])
                self.vtiles[g, c] = ts.sum()
        # idx array columns (16-wrapped): per (g, c) slice offsets
        self.idx_cols = self.vtiles * 8                        # tiles*128/16
        self.idx_off = np.zeros((G, cfg.nchunk), np.int64)
        self.idx_off[:, 1:] = np.cumsum(self.idx_cols, axis=1)[:, :-1]
        self.idx_cols_total = int(self.idx_cols.sum(axis=1).max())
        # operand (dstl/w) columns: block-major within a group
        self.op_off = np.zeros(cfg.blocks_per_core, np.int64)
        for g in range(G):
            nt = self.NTb[g * B:(g + 1) * B]
            self.op_off[g * B:(g + 1) * B] = (
                np.concatenate([[0], np.cumsum(nt)[:-1]]))
        self.op_cols_total = int(
            self.NTb.reshape(G, B).sum(axis=1).max())
        self.vtiles_max = int(self.vtiles.max())
        self.NTb_max = int(self.NTb.max())

    def key(self):
        return self.Tmat.tobytes()


def preprocess(cfg: Cfg, x, src, dst, batch):
    """Compute the node permutation, normalization constants and the padded
    per-core edge streams.  Returns (tc, per_core_meta, shared)."""
    N, E, P = cfg.n_nodes, cfg.n_edges, cfg.p
    NB = P * cfg.blocks_per_core              # global block count
    CH = cfg.chunk

    deg = np.bincount(dst, minlength=N).astype(np.float64) + 1.0
    dinv = 1.0 / np.sqrt(deg)
    norm = (dinv[src] * dinv[dst]).astype(np.float64)
    dinv2 = dinv * dinv
    nsum = np.bincount(dst, weights=norm, minlength=N) + dinv2

    # --- serpentine degree-balanced assignment of nodes to global blocks
    order = np.argsort(-deg, kind="stable")
    k = np.arange(N)
    rowi, coli = k // NB, k % NB
    blk_ser = np.where(rowi % 2 == 0, coli, NB - 1 - coli)
    blk_of = np.empty(N, np.int64)
    blk_of[order] = blk_ser
    # slot index within block: stable order of appearance
    o2 = np.argsort(blk_of, kind="stable")
    slot_idx = np.empty(N, np.int64)
    counts = np.bincount(blk_of, minlength=NB)
    assert counts.max() <= 128
    starts = np.concatenate([[0], np.cumsum(counts)[:-1]])
    slot_idx[o2] = np.arange(N) - starts[blk_of[o2]]
    row_of = blk_of * 128 + slot_idx                  # table row of each node

    # --- rank-align block positions across cores so the per-position
    # max-over-cores tile count (the SPMD padding) is tight: within each
    # core, order blocks by total load.  A core's blocks all live in one
    # chunk, so relabeling within a core never moves a node across chunks.
    load = np.zeros(NB, np.int64)
    np.add.at(load, blk_of[dst], 1)
    load += np.bincount(blk_of, minlength=NB)          # self loops
    lb2 = np.empty(NB, np.int64)
    for r in range(P):
        lo = load[r * cfg.blocks_per_core:(r + 1) * cfg.blocks_per_core]
        rank = np.empty(cfg.blocks_per_core, np.int64)
        rank[np.argsort(-lo, kind="stable")] = np.arange(cfg.blocks_per_core)
        lb2[r * cfg.blocks_per_core:(r + 1) * cfg.blocks_per_core] = (
            r * cfg.blocks_per_core + rank)
    blk_of = lb2[blk_of]
    row_of = blk_of * 128 + slot_idx

    # --- edge items (graph edges + self loops), keyed by (block, chunk)
    eb = blk_of[dst]
    ei = row_of[src]
    items_key = eb * cfg.nchunk + (ei // CH)
    items_idx = (ei % CH).astype(np.int64)
    items_dstl = (row_of[dst] % 128).astype(np.int64)
    items_w = norm

    sb = blk_of  # self loops: one per node
    si = row_of
    skey = sb * cfg.nchunk + (si // CH)
    key = np.concatenate([items_key, skey])
    idxv = np.concatenate([items_idx, si % CH])
    dstlv = np.concatenate([items_dstl, row_of % 128])
    wv = np.concatenate([items_w, dinv2])

    nbuck = NB * cfg.nchunk
    cnt = np.bincount(key, minlength=nbuck)

    # per-(local block, chunk) tile counts, maxed across cores so the SPMD
    # program structure is uniform
    cnt3 = cnt.reshape(P, cfg.blocks_per_core, cfg.nchunk)
    Tmat = np.ceil(cnt3.max(axis=0) / 128).astype(np.int64)   # [BPC, nchunk]
    Tmat = np.maximum(Tmat, 1)
    lay = Layout(cfg, Tmat)

    # position of each item within its bucket
    o3 = np.argsort(key, kind="stable")
    bstart = np.concatenate([[0], np.cumsum(cnt)[:-1]])
    pos = np.arange(key.size) - bstart[key[o3]]

    # scatter items into the padded per-bucket layout (flat address space)
    cap_flat = np.tile(lay.cap.reshape(-1), P)           # [nbuck]
    boff = np.concatenate([[0], np.cumsum(cap_flat)[:-1]])
    addr = boff[key[o3]] + pos
    tot = int(cap_flat.sum())
    flat_idx = np.zeros(tot, np.int16)
    flat_dstl = np.full(tot, -1.0, np.float32)
    flat_w = np.zeros(tot, np.float32)
    flat_idx[addr] = idxv[o3].astype(np.int16)
    flat_dstl[addr] = dstlv[o3].astype(np.float32)
    flat_w[addr] = wv[o3].astype(np.float32)
    per_core_tot = tot // P
    flat_idx = flat_idx.reshape(P, per_core_tot)
    flat_dstl = flat_dstl.reshape(P, per_core_tot)
    flat_w = flat_w.reshape(P, per_core_tot)

    # bucket start offsets within one core's flat stream, [BPC, nchunk]
    bko = boff[:nbuck // P].reshape(cfg.blocks_per_core, cfg.nchunk)

    per_core = []
    for r in range(P):
        b0 = r * cfg.blocks_per_core
        b1 = b0 + cfg.blocks_per_core
        # gather index streams: [groups, 128, sum(call cols)] int16
        gi = np.zeros((cfg.groups, 128, lay.idx_cols_total), np.int16)
        dl = np.full((cfg.groups, 128, lay.op_cols_total), -1.0, np.float32)
        wl_ = np.zeros((cfg.groups, 128, lay.op_cols_total), np.float32)
        for g in range(cfg.groups):
            for c in range(cfg.nchunk):
                parts = []
                for bl in range(cfg.bpg):
                    b = g * cfg.bpg + bl
                    o = bko[b, c]
                    parts.append(flat_idx[r, o:o + lay.cap[b, c]])
                stream = np.concatenate(parts)
                w16 = stream.reshape(-1, 16).T            # wrap 16 partitions
                j0 = lay.idx_off[g][c]
                gi[g, :, j0:j0 + w16.shape[1]] = np.tile(w16, (8, 1))
            for bl in range(cfg.bpg):
                b = g * cfg.bpg + bl
                for c in range(cfg.nchunk):
                    o = bko[b, c]
                    T = lay.Tmat[b, c]
                    blk_d = flat_dstl[r, o:o + T * 128].reshape(T, 128).T
                    blk_w = flat_w[r, o:o + T * 128].reshape(T, 128).T
                    j0 = lay.op_off[b] + lay.coff[b, c]
                    dl[g, :, j0:j0 + T] = blk_d
                    wl_[g, :, j0:j0 + T] = blk_w
        wl_ = wl_.astype(BF16)

        # batch_local [128, BPC] and nsum_ones [2, SLOTS]
        bl = np.full((cfg.blocks_per_core, 128), -1.0, np.float32)
        ns = np.zeros((cfg.blocks_per_core, 128), np.float32)
        nodes_r = np.where((blk_of >= b0) & (blk_of < b1))[0]
        lb = blk_of[nodes_r] - b0
        sl = slot_idx[nodes_r]
        bl[lb, sl] = batch[nodes_r].astype(np.float32)
        ns[lb, sl] = nsum[nodes_r].astype(np.float32)
        batchl = bl.T.copy()                          # [128, BPC]
        nsum1 = np.stack([ns.reshape(-1),
                          np.ones(cfg.slots, np.float32)])  # [2, SLOTS]
        per_core.append(dict(idx=gi, dstl=dl, wgt=wl_, batchl=batchl,
                             nsum1=nsum1))

    xt = np.zeros((cfg.rows, cfg.d), BF16)
    xt[row_of] = x.astype(BF16)
    return lay, per_core, dict(xt=xt, row_of=row_of)


def make_consts(cfg: Cfg, W_enc, b_enc, W1, b1, W2, b2, W3, b3, Wr1, br1,
                Wr2, br2):
    d = cfg.d
    wf1 = (W_enc.astype(np.float64) @ W1.astype(np.float64)).astype(F32)
    bw = (b_enc.astype(np.float64) @ W1.astype(np.float64)).astype(F32)
    wl = np.stack([wf1, W2.astype(F32), W3.astype(F32)])          # [3,d,d]
    brhs = np.zeros((3, 2, d), F32)
    brhs[0, 0] = bw
    brhs[0, 1] = b1
    brhs[1, 1] = b2
    brhs[2, 1] = b3
    iota128 = np.tile(np.arange(128, dtype=F32), (128, 1))
    iotag = np.tile(np.arange(cfg.n_graphs, dtype=F32), (128, 1))
    return dict(
        wl=wl, brhs=brhs, iota128=iota128, iotag=iotag,
        wr1=Wr1.astype(F32), br1=br1.astype(F32).reshape(d, 1),
        wr2=Wr2.astype(F32), br2=br2.astype(F32).reshape(cfg.n_classes, 1),
    )


# ---------------------------------------------------------------- device side

def build_program(cfg: Cfg, lay: Layout, enable_asserts=False, debug=False,
                  maxt=8, no_gather=False, nq=4):
    d = cfg.d
    G = cfg.n_graphs
    BPC, GR, BPGR = cfg.blocks_per_core, cfg.groups, cfg.bpg
    Tm, coff, NTb = lay.Tmat, lay.coff, lay.NTb

    nc = bacc.Bacc("TRN2", target_bir_lowering=False, debug=debug,
                   enable_asserts=enable_asserts, num_devices=cfg.p,
                   num_swdge_queues=nq)

    ti = lambda n, s, t: nc.dram_tensor(n, s, t, kind="ExternalInput")
    xt_t = ti("xt", [cfg.rows, d], DT.bfloat16)
    idx_t = ti("idx", [GR, 128, lay.idx_cols_total], DT.int16)
    dstl_t = ti("dstl", [GR, 128, lay.op_cols_total], DT.float32)
    wgt_t = ti("wgt", [GR, 128, lay.op_cols_total], DT.bfloat16)
    batchl_t = ti("batchl", [128, BPC], DT.float32)
    nsum1_t = ti("nsum1", [2, cfg.slots], DT.float32)
    wl_t = ti("wl", [3, d, d], DT.float32)
    brhs_t = ti("brhs", [3, 2, d], DT.float32)
    iota128_t = ti("iota128", [128, 128], DT.float32)
    iotag_t = ti("iotag", [128, G], DT.float32)
    wr1_t = ti("wr1", [d, d], DT.float32)
    br1_t = ti("br1", [d, 1], DT.float32)
    wr2_t = ti("wr2", [d, cfg.n_classes], DT.float32)
    br2_t = ti("br2", [cfg.n_classes, 1], DT.float32)

    logits_t = nc.dram_tensor("logits", [cfg.n_classes, G], DT.float32,
                              kind="ExternalOutput")

    # internal DRAM
    shard = [nc.dram_tensor(f"shard{l}", [cfg.slots, d], DT.bfloat16)
             for l in range(2)]
    tbl = [nc.dram_tensor(f"tbl{l}", [cfg.rows, d], DT.bfloat16,
                          addr_space="Shared") for l in range(2)]
    pool_in = nc.dram_tensor("pool_in", [d, G], DT.float32)
    pool_out = nc.dram_tensor("pool_out", [d, G], DT.float32,
                              addr_space="Shared")

    rg = [list(range(cfg.p))]

    def bcast3(ap2, reps, inner):
        """[P, C] AP -> [P, C, inner] AP with a step-0 inner dim (reps = C)."""
        a = ap2
        assert len(a.ap) == 2
        return AP(a.tensor, a.offset, [a.ap[0], [a.ap[1][0], reps], [0, inner]])

    with tile.TileContext(nc) as tc:
        with (
            tc.tile_pool(name="const", bufs=1) as cpool,
            tc.tile_pool(name="meta", bufs=3) as mpool,
            tc.tile_pool(name="v0", bufs=2) as vp0,
            tc.tile_pool(name="v1", bufs=2) as vp1,
            tc.tile_pool(name="v2", bufs=2) as vp2,
            tc.tile_pool(name="v3", bufs=2) as vp3,
            tc.tile_pool(name="s", bufs=2) as spool,
            tc.tile_pool(name="work", bufs=2) as wpool,
            tc.tile_pool(name="rout", bufs=1) as routp,
            tc.tile_pool(name="agg", bufs=2, space="PSUM") as aggp,
            tc.tile_pool(name="hps", bufs=2, space="PSUM") as hpsp,
            tc.tile_pool(name="poolacc", bufs=1, space="PSUM") as plp,
            tc.tile_pool(name="ro", bufs=1, space="PSUM") as rop,
        ):
            vpools = [vp0, vp1, vp2, vp3]

            # ---- resident constants / metadata
            iota = cpool.tile([128, 128], DT.float32)
            nc.sync.dma_start(iota[:], iota128_t[:])
            iotag = cpool.tile([128, G], DT.float32)
            nc.sync.dma_start(iotag[:], iotag_t[:])
            wl = cpool.tile([128, 3, d], DT.float32)
            nc.sync.dma_start(wl[:], wl_t.rearrange("l k j -> k l j"))
            brhs = cpool.tile([2, 3, d], DT.float32)
            nc.sync.dma_start(brhs[:], brhs_t.rearrange("l k j -> k l j"))
            wr1 = cpool.tile([d, d], DT.float32)
            nc.sync.dma_start(wr1[:], wr1_t[:])
            br1 = cpool.tile([d, 1], DT.float32)
            nc.sync.dma_start(br1[:], br1_t[:])
            wr2 = cpool.tile([d, cfg.n_classes], DT.float32)
            nc.sync.dma_start(wr2[:], wr2_t[:])
            br2 = cpool.tile([cfg.n_classes, 1], DT.float32)
            nc.sync.dma_start(br2[:], br2_t[:])
            batchl = cpool.tile([128, BPC], DT.float32)
            nc.sync.dma_start(batchl[:], batchl_t[:])
            nsum1 = cpool.tile([2, cfg.slots], DT.float32)
            nc.sync.dma_start(nsum1[:], nsum1_t[:])

            pool_ps = plp.tile([d, G], DT.float32)

            for l in range(3):
                table = xt_t if l == 0 else tbl[l - 1]
                for g in range(GR):
                    idxg = mpool.tile([128, lay.idx_cols_total], DT.int16,
                                      tag="idxg")
                    nc.sync.dma_start(idxg[:], idx_t[g])
                    dstlg = mpool.tile([128, lay.op_cols_total], DT.float32,
                                       tag="dstlg")
                    nc.sync.dma_start(dstlg[:], dstl_t[g])
                    wgtg = mpool.tile([128, lay.op_cols_total], DT.bfloat16,
                                      tag="wgtg")
                    nc.sync.dma_start(wgtg[:], wgt_t[g])
                    vt = []
                    for c in range(cfg.nchunk):
                        nv = int(lay.vtiles[g, c])
                        v = vpools[c].tile([128, lay.vtiles_max, d],
                                           DT.bfloat16, tag=f"v{c}")
                        i0 = int(lay.idx_off[g, c])
                        if no_gather:
                            nc.vector.memset(v[:], 0.5)
                        # SWDGE descriptor-ring carveout caps one gather at
                        # <4096 indices; split into sub-calls of <=maxt tiles.
                        for j0 in range(0, 0 if no_gather else nv, maxt):
                            j1 = min(j0 + maxt, nv)
                            nc.gpsimd.dma_gather(
                                out_ap=v[:, j0:j1, :],
                                in_ap=table[c * cfg.chunk:
                                            (c + 1) * cfg.chunk, :],
                                idxs_ap=idxg[:, i0 + j0 * 8:i0 + j1 * 8],
                                num_idxs=(j1 - j0) * 128,
                                num_idxs_reg=(j1 - j0) * 128,
                                elem_size=d,
                                queue_num=c % nq,
                            )
                        vt.append(v)
                    for bl in range(BPGR):
                        b = g * BPGR + bl
                        nt = int(NTb[b])
                        col0 = int(lay.op_off[b])
                        # S = (iota == dstl) * w   [128, nt*128] bf16
                        s01 = spool.tile([128, lay.NTb_max * 128],
                                         DT.bfloat16, tag="s01")
                        s = spool.tile([128, lay.NTb_max * 128],
                                       DT.bfloat16, tag="s")
                        io3 = AP(iota[:].tensor, iota[:].offset,
                                 [iota[:].ap[0], [0, nt], iota[:].ap[1]])
                        d2 = dstlg[:, col0:col0 + nt]
                        w2 = wgtg[:, col0:col0 + nt]
                        s3 = s[:, :nt * 128].rearrange(
                            "p (t j) -> p t j", j=128)
                        s013 = s01[:, :nt * 128].rearrange(
                            "p (t j) -> p t j", j=128)
                        nc.vector.tensor_tensor(
                            out=s013, in0=io3, in1=bcast3(d2, nt, 128),
                            op=mybir.AluOpType.is_equal)
                        nc.vector.tensor_tensor(
                            out=s3, in0=s013, in1=bcast3(w2, nt, 128),
                            op=mybir.AluOpType.mult)
                        agg = aggp.tile([d, 128], DT.float32)
                        for c in range(cfg.nchunk):
                            for t in range(int(Tm[b, c])):
                                kk = int(coff[b, c]) + t
                                nc.tensor.matmul(
                                    agg[:],
                                    lhsT=vt[c][:, int(lay.voff[g, c, bl]) + t, :],
                                    rhs=s[:, kk * 128:(kk + 1) * 128],
                                    start=(kk == 0), stop=(kk == nt - 1))
                        aggs = wpool.tile([d, 128], DT.float32, tag="aggs")
                        nc.scalar.activation(aggs[:], agg[:],
                                             mybir.ActivationFunctionType.Copy)
                        hps = hpsp.tile([128, d], DT.float32)
                        nc.tensor.matmul(hps[:], lhsT=aggs[:],
                                         rhs=wl[:, l, :],
                                         start=True, stop=False)
                        nc.tensor.matmul(
                            hps[:], lhsT=nsum1[:, b * 128:(b + 1) * 128],
                            rhs=brhs[:, l, :], start=False, stop=True)
                        if l < 2:
                            hout = wpool.tile([128, d], DT.bfloat16,
                                              tag="hout")
                            nc.scalar.activation(
                                hout[:], hps[:],
                                mybir.ActivationFunctionType.Relu)
                            nc.sync.dma_start(
                                shard[l][b * 128:(b + 1) * 128, :], hout[:])
                        else:
                            h3 = wpool.tile([128, d], DT.float16, tag="h3")
                            nc.scalar.activation(
                                h3[:], hps[:],
                                mybir.ActivationFunctionType.Relu)
                            sp = spool.tile([128, G], DT.float16, tag="sp")
                            nc.vector.tensor_scalar(
                                out=sp[:], in0=iotag[:],
                                scalar1=batchl[:, b:b + 1], scalar2=None,
                                op0=mybir.AluOpType.is_equal)
                            nc.tensor.matmul(pool_ps[:], lhsT=h3[:],
                                             rhs=sp[:], start=(b == 0),
                                             stop=(b == BPC - 1))
                if l < 2:
                    nc.gpsimd.collective_compute(
                        "AllGather", mybir.AluOpType.bypass,
                        replica_groups=rg,
                        ins=[shard[l][:]], outs=[tbl[l][:]])

            # ---- pooled readout
            pools = routp.tile([d, G], DT.float32, tag="pools")
            nc.scalar.activation(pools[:], pool_ps[:],
                                 mybir.ActivationFunctionType.Copy)
            nc.sync.dma_start(pool_in[:], pools[:])
            nc.gpsimd.collective_compute(
                "AllReduce", mybir.AluOpType.add, replica_groups=rg,
                ins=[pool_in[:]], outs=[pool_out[:]])
            gfm = routp.tile([d, G], DT.float32, tag="gfm")
            nc.sync.dma_start(gfm[:], pool_out[:])
            z1p = rop.tile([d, G], DT.float32, tag="ro")
            nc.tensor.matmul(z1p[:], lhsT=wr1[:], rhs=gfm[:],
                             start=True, stop=True)
            z1 = routp.tile([d, G], DT.float32, tag="z1")
            nc.scalar.activation(z1[:], z1p[:],
                                 mybir.ActivationFunctionType.Relu,
                                 bias=br1[:])
            lgp = rop.tile([cfg.n_classes, G], DT.float32, tag="ro")
            nc.tensor.matmul(lgp[:], lhsT=wr2[:], rhs=z1[:],
                             start=True, stop=True)
            lg = routp.tile([cfg.n_classes, G], DT.float32, tag="lg")
            nc.scalar.activation(lg[:], lgp[:],
                                 mybir.ActivationFunctionType.Identity,
                                 bias=br2[:])
            nc.sync.dma_start(logits_t[:], lg[:])

    nc.compile()
    return nc


# ---------------------------------------------------------------- entry point

_PROG_CACHE = {}


def _get_program(cfg, lay, **kw):
    key = (cfg, lay.key(), tuple(sorted(kw.items())))
    if key not in _PROG_CACHE:
        _PROG_CACHE[key] = build_program(cfg, lay, **kw)
    return _PROG_CACHE[key]


def make_in_maps(cfg, inputs):
    x = np.asarray(inputs["x"], np.float32)
    src = np.asarray(inputs["c_2"], np.int64)
    dst = np.asarray(inputs["u_2"], np.int64)
    batch = np.asarray(inputs["batch"], np.int64)
    lay, per_core, shared = preprocess(cfg, x, src, dst, batch)
    consts = make_consts(
        cfg, *(np.asarray(inputs[k], np.float32) for k in
               ("W_enc", "b_enc", "W1", "b1", "W2", "b2", "W3", "b3",
                "Wr1", "br1", "Wr2", "br2")))
    in_maps = []
    for r in range(cfg.p):
        m = dict(per_core[r])
        m["xt"] = shared["xt"]
        m.update(consts)
        in_maps.append(m)
    return lay, in_maps


def kernel(**inputs) -> np.ndarray:
    cfg = REAL_CFG
    lay, in_maps = make_in_maps(cfg, inputs)
    nc = _get_program(cfg, lay)
    res = run_bass_kernel_spmd(nc, in_maps, list(range(cfg.p)))
    logits = np.asarray(res.results[0]["logits"], np.float32)
    return np.ascontiguousarray(logits.T)
